# revision 17
# baseline (speedup 1.0000x reference)
# GNN mean-aggregation kernel for Trainium2 (8 NeuronCores, SPMD).
#
# Computes: out[i] = (1/deg_i) * sum_{(i,j) in E} (x[j] @ W + b)
# using the identity out = inv_deg * (A @ x) @ W + b*mask, so the dense
# linear layer runs once on the aggregated rows instead of per edge.
#
# Sharding: destination nodes (and their incoming edge rows -- `row` is
# sorted) are split contiguously across 8 cores; x and W are replicated,
# so no collectives are needed.
#
# Per-core pipeline (all payload data in bf16; PSUM accumulates fp32):
#   1. dma_gather (GPSIMD SWDGE) fetches x[col] rows (256B bf16 each)
#      from HBM, one call per (8-tile group, 25k-row source chunk), so
#      the ~1us fixed SWDGE cost amortizes over 8k indices.  int16
#      gather indices only span 32k rows, so x is addressed in 4 chunks
#      of 25k rows.  Edges are host-binned by (dest-tile, chunk) with a
#      vector bin-packing pass that keeps every bin <= C_sub*128 slots.
#   2. DVE builds one-hot segment matrices S^T[e,d] = (rel[e]==d) from
#      host-provided relative-dest values via tensor_tensor(is_equal).
#   3. PE accumulates AGG^T = sum_j M_j^T @ S^T_j in PSUM per 128-dest
#      tile (bf16 matmuls, 1 cycle/row), then OUT^T = W^T @ AGG^T
#      + b (x) deg  (rank-1 bias matmul).
#   4. DVE scales by inv_deg along the dest axis; DMA writes OUT^T fp32.
# Host post-processing transposes and concatenates the per-core outputs.

import math

import numpy as np

P = 128
F = 128


class _Cfg:
    def __init__(self, n_nodes, n_cores, n_chunks, n_tiles, group_tiles=8):
        self.NN = n_nodes
        self.NCORES = n_cores
        self.NDEST = n_nodes // n_cores
        self.NT = n_tiles
        assert self.NT * P >= self.NDEST
        self.NCH = n_chunks
        self.CH = math.ceil(n_nodes / n_chunks)
        assert self.CH <= 32768
        self.G = group_tiles


CFG = _Cfg(100000, 8, 4, 100, group_tiles=4)

_BUILD_CACHE = {}


def _pack_tiles(cfg, cnt):
    """Assign NDEST dests to NT tiles (<=128 each) minimizing the max
    per-(tile, chunk) edge count.  cnt: [NDEST, NCH] int per-chunk degree."""
    NT, NCH = cfg.NT, cfg.NCH
    NDEST = cfg.NDEST
    order_d = np.argsort(-cnt.max(axis=1), kind="stable")
    sums = np.zeros((NT, NCH), np.int64)
    counts = np.zeros(NT, np.int64)
    pos = np.empty(NDEST, np.int64)
    big = np.int64(1) << 40
    for d in order_d:
        score = (sums + cnt[d]).max(axis=1)
        score[counts >= P] = big
        t = int(np.argmin(score))
        pos[d] = t * P + counts[t]
        counts[t] += 1
        sums[t] += cnt[d]
    return pos


def _host_prep(cfg, x, row, col, W, b):
    NN, NCORES, NDEST, NT, NCH, CH = (
        cfg.NN, cfg.NCORES, cfg.NDEST, cfg.NT, cfg.NCH, cfg.CH)
    NE = row.shape[0]
    row = np.asarray(row).astype(np.int64)
    col = np.asarray(col).astype(np.int64)
    x = np.ascontiguousarray(np.asarray(x, dtype=np.float32))
    W = np.ascontiguousarray(np.asarray(W, dtype=np.float32))
    b = np.asarray(b, dtype=np.float32)

    deg = np.bincount(row, minlength=NN).astype(np.float32)
    invdeg = np.where(deg > 0, 1.0 / np.maximum(deg, 1.0), 0.0).astype(np.float32)

    core = row // NDEST
    r_in_core = row % NDEST
    chunk = col // CH
    idx16 = (col % CH).astype(np.int16)

    # perm[core, d_local] = permuted position (tile*128 + slot).
    perm = np.zeros((NCORES, NDEST), np.int64)
    for c in range(NCORES):
        m = core == c
        key = r_in_core[m] * NCH + chunk[m]
        cnt = np.bincount(key, minlength=NDEST * NCH).reshape(NDEST, NCH)
        perm[c] = _pack_tiles(cfg, cnt)
    tilei = perm[core, r_in_core] // P
    rel = (perm[core, r_in_core] % P).astype(np.float32)

    bin_key = (core * NT + tilei) * NCH + chunk
    nbins = NCORES * NT * NCH
    counts = np.bincount(bin_key, minlength=nbins)
    C_sub = max(1, int(math.ceil(counts.max() / P)))
    SLOT = C_sub * P

    order = np.argsort(bin_key, kind="stable")
    sk = bin_key[order]
    starts = np.concatenate([[0], np.cumsum(counts)[:-1]])
    rank = np.arange(NE, dtype=np.int64) - starts[sk]
    pos = sk * SLOT + rank

    # pads get idx=-1 when bins align with the 1024-idx gather calls
    # (pads then trail within their call and the SWDGE ucode trims them,
    # generating no descriptors).  Mid-call negatives would fault, so use
    # idx=0 when calls cross bin boundaries.
    TOT = nbins * SLOT
    pad_idx = -1 if SLOT % 1024 == 0 else 0
    idx_pad = np.full(TOT, pad_idx, np.int16)
    rel_pad = np.full(TOT, -1.0, np.float32)
    idx_pad[pos] = idx16[order]
    rel_pad[pos] = rel[order]
    idx_pad = idx_pad.reshape(NCORES, NT, NCH, SLOT)
    rel_pad = rel_pad.reshape(NCORES, NT, NCH, C_sub, P)

    groups = [(t0, min(t0 + cfg.G, NT)) for t0 in range(0, NT, cfg.G)]

    import ml_dtypes
    bf16 = np.dtype(ml_dtypes.bfloat16)
    x_bf = np.ascontiguousarray(x.astype(bf16))
    W_bf = np.ascontiguousarray(W.astype(bf16))
    iota2 = np.tile(np.arange(P, dtype=np.float32)[None, :], (P, 1)).astype(bf16)
    brow = b[None, :].astype(bf16)

    in_maps = []
    for c in range(NCORES):
        # gather-call index stream: per (group, chunk), wrapped per
        # <=1024-idx call (SWDGE descriptor ring holds ~128 descs/queue;
        # a call generates num_idxs/16+1 per DMA engine), 16-wrapped and
        # replicated to 128 partitions.
        wrapped_parts = []
        for (t0, t1) in groups:
            for ch in range(NCH):
                seq = idx_pad[c, t0:t1, ch].reshape(-1)
                for k0 in range(0, len(seq), 1024):
                    seg = seq[k0:k0 + 1024]
                    wrapped_parts.append(np.tile(seg.reshape(-1, 16).T, (8, 1)))
        idx_t = np.concatenate(wrapped_parts, axis=1)

        # rel layout [P, (ch, t, j)] so a (group, chunk) slice is contiguous
        rel_t = np.ascontiguousarray(
            rel_pad[c].transpose(3, 1, 0, 2).reshape(P, NCH * NT * C_sub)
        ).astype(bf16)

        dsl = slice(c * NDEST, (c + 1) * NDEST)
        ivc = np.zeros(NT * P, np.float32)
        ivc[perm[c]] = invdeg[dsl]
        dgc = np.zeros(NT * P, np.float32)
        dgc[perm[c]] = deg[dsl]

        in_maps.append({
            "x": x_bf,
            "idxs": np.ascontiguousarray(idx_t),
            "rel": rel_t,
            "invdeg": np.ascontiguousarray(np.tile(ivc[None, :], (P, 1))),
            "degr": dgc[None, :].astype(bf16),
            "w": W_bf,
            "brow": brow,
            "iota2": iota2,
        })
    return C_sub, in_maps, perm


def _build(cfg, C_sub, repeat):
    import concourse.mybir as mybir
    import concourse.tile as tile
    from concourse import bacc

    f32 = mybir.dt.float32
    bf16 = mybir.dt.bfloat16
    i16 = mybir.dt.int16
    eq = mybir.AluOpType.is_equal
    mult = mybir.AluOpType.mult

    NT, NCH, CH, G = cfg.NT, cfg.NCH, cfg.CH, cfg.G
    IDXW = NT * NCH * C_sub * P // 16

    nc = bacc.Bacc("TRN2", debug=False, num_swdge_queues=4)
    x_d = nc.dram_tensor("x", [cfg.NN, F], bf16, kind="ExternalInput")
    idx_d = nc.dram_tensor("idxs", [P, IDXW], i16, kind="ExternalInput")
    rel_d = nc.dram_tensor("rel", [P, NCH * NT * C_sub], bf16, kind="ExternalInput")
    invdeg_d = nc.dram_tensor("invdeg", [P, NT * P], f32, kind="ExternalInput")
    deg_d = nc.dram_tensor("degr", [1, NT * P], bf16, kind="ExternalInput")
    w_d = nc.dram_tensor("w", [F, F], bf16, kind="ExternalInput")
    b_d = nc.dram_tensor("brow", [1, F], bf16, kind="ExternalInput")
    iota_d = nc.dram_tensor("iota2", [P, P], bf16, kind="ExternalInput")
    out_d = nc.dram_tensor("outT", [P, NT * P], f32, kind="ExternalOutput")

    groups = [(t0, min(t0 + G, NT)) for t0 in range(0, NT, G)]
    x_ap = x_d.ap()

    with tile.TileContext(nc) as tc:
        with (
            tc.tile_pool(name="const", bufs=1) as constp,
            tc.tile_pool(name="reg", bufs=2) as regionp,
            tc.tile_pool(name="st", bufs=2) as stp,
            tc.tile_pool(name="idx", bufs=2) as idxp,
            tc.tile_pool(name="small", bufs=8) as smallp,
            tc.tile_pool(name="grp", bufs=2) as grpp,
            # PSUM pools are bank-granular (8 banks of 2KB, one acc per
            # bank); with G=4 tiles/group, bufs=8 double-buffers two
            # groups of accumulators so group boundaries don't stall PE.
            tc.tile_pool(name="acc", bufs=8, space="PSUM") as accp,
        ):
            w_sb = constp.tile([F, F], bf16)
            nc.sync.dma_start(w_sb[:], w_d.ap())
            b_sb = constp.tile([1, F], bf16)
            nc.sync.dma_start(b_sb[:], b_d.ap())
            iota_sb = constp.tile([P, P], bf16)
            nc.sync.dma_start(iota_sb[:], iota_d.ap())
            rel_sb = constp.tile([P, NCH * NT * C_sub], bf16)
            nc.sync.dma_start(rel_sb[:], rel_d.ap())

            # Zero both reg pool buffers once: gather calls skip trailing
            # pad slots (idx=-1), leaving stale SBUF bytes there.  After
            # this, stale bytes are always finite x rows from a previous
            # batch (the one-hot zeroes their contribution, but NaN*0=NaN,
            # so the bytes must at least be finite).
            ncols0 = min(G, NT) * C_sub
            for _ in range(2):
                rz = regionp.tile([P, ncols0, P], bf16, tag="reg")
                nc.vector.memset(rz[:], 0.0)

            def body(_iv=None):
                idx_off = 0
                qn = 0
                for (t0, t1) in groups:
                    gt = t1 - t0
                    ncols = gt * C_sub
                    L = ncols * P
                    invdeg_g = grpp.tile([P, gt * P], f32, tag="invdeg")
                    nc.sync.dma_start(
                        invdeg_g[:], invdeg_d.ap()[:, t0 * P:t1 * P])
                    deg_g = grpp.tile([1, gt * P], bf16, tag="deg")
                    nc.sync.dma_start(deg_g[:], deg_d.ap()[:, t0 * P:t1 * P])
                    accs = [
                        accp.tile([P, P], f32, tag="acc",
                                  name=f"acc{t0}_{k}")[:]
                        for k in range(gt)
                    ]
                    for ch in range(NCH):
                        reg = regionp.tile([P, ncols, P], bf16, tag="reg")
                        idxt = idxp.tile([P, L // 16], i16, tag="idx")
                        nc.sync.dma_start(
                            idxt[:], idx_d.ap()[:, idx_off:idx_off + L // 16])
                        idx_off += L // 16
                        for k0 in range(0, ncols, 8):
                            kc = min(8, ncols - k0)
                            Lk = kc * P
                            nc.gpsimd.dma_gather(
                                out_ap=reg[:, k0:k0 + kc, :],
                                in_ap=x_ap[ch * CH:(ch + 1) * CH, :],
                                idxs_ap=idxt[:, k0 * 8:k0 * 8 + kc * 8],
                                num_idxs=Lk,
                                num_idxs_reg=Lk,
                                elem_size=F,
                                queue_num=qn % 4,
                            )
                            qn += 1
                        st = stp.tile([P, ncols, P], bf16, tag="st")
                        base = ch * NT * C_sub
                        rel_sl = rel_sb[:, base + t0 * C_sub:base + t1 * C_sub]
                        nc.vector.tensor_tensor(
                            out=st[:],
                            in0=iota_sb[:].unsqueeze(1).to_broadcast(
                                [P, ncols, P]),
                            in1=rel_sl.to_broadcast([P, ncols, P]),
                            op=eq,
                        )
                        for ti in range(gt):
                            accap = accs[ti]
                            for j in range(C_sub):
                                k = ti * C_sub + j
                                nc.tensor.matmul(
                                    out=accap,
                                    lhsT=reg[:, k, :],
                                    rhs=st[:, k, :],
                                    start=(ch == 0 and j == 0),
                                    stop=(ch == NCH - 1 and j == C_sub - 1),
                                )
                    for ti in range(gt):
                        t = t0 + ti
                        accap = accs[ti]
                        aggT = smallp.tile([P, P], bf16, tag="agg")
                        nc.scalar.copy(aggT[:], accap)
                        # reuse the same PSUM bank for the output matmul
                        nc.tensor.matmul(out=accap, lhsT=w_sb[:],
                                         rhs=aggT[:], start=True, stop=False)
                        nc.tensor.matmul(out=accap, lhsT=b_sb[:1, :],
                                         rhs=deg_g[:1, ti * P:(ti + 1) * P],
                                         start=False, stop=True)
                        osb = smallp.tile([P, P], f32, tag="osb")
                        nc.vector.tensor_tensor(
                            out=osb[:], in0=accap,
                            in1=invdeg_g[:, ti * P:(ti + 1) * P], op=mult)
                        nc.sync.dma_start(
                            out_d.ap()[:, t * P:(t + 1) * P], osb[:])

            if repeat == 1:
                body()
            else:
                with tc.For_i(0, repeat, 1) as iv:
                    body(iv)

    nc.compile()
    return nc


def _run(cfg, x, row, col, W, b, repeat=1, core_ids=None):
    from concourse import bass_utils

    C_sub, in_maps, perm = _host_prep(cfg, x, row, col, W, b)
    key = (cfg.NN, cfg.NCORES, C_sub, repeat)
    if key not in _BUILD_CACHE:
        _BUILD_CACHE[key] = _build(cfg, C_sub, repeat)
    nc = _BUILD_CACHE[key]
    if core_ids is None:
        core_ids = list(range(cfg.NCORES))
    res = bass_utils.run_bass_kernel_spmd(nc, in_maps, core_ids=core_ids)
    outs = []
    for c in range(len(core_ids)):
        outT = res.results[c]["outT"]
        outs.append(outT.T[perm[c]].astype(np.float32))
    return np.concatenate(outs, axis=0)


def kernel(x, row, col, W, b):
    return _run(CFG, x, row, col, W, b, repeat=1)


# revision 19
# speedup vs baseline: 1.0187x; 1.0187x over previous
# GNN mean-aggregation kernel for Trainium2 (8 NeuronCores, SPMD).
#
# Computes: out[i] = (1/deg_i) * sum_{(i,j) in E} (x[j] @ W + b)
# using the identity out = inv_deg * (A @ x) @ W + b*mask, so the dense
# linear layer runs once on the aggregated rows instead of per edge.
#
# Sharding: destination nodes (and their incoming edge rows -- `row` is
# sorted) are split contiguously across 8 cores; x and W are replicated,
# so no collectives are needed.
#
# Per-core pipeline (all payload data in bf16; PSUM accumulates fp32):
#   1. dma_gather (GPSIMD SWDGE) fetches x[col] rows (256B bf16 each)
#      from HBM, one call per (8-tile group, 25k-row source chunk), so
#      the ~1us fixed SWDGE cost amortizes over 8k indices.  int16
#      gather indices only span 32k rows, so x is addressed in 4 chunks
#      of 25k rows.  Edges are host-binned by (dest-tile, chunk) with a
#      vector bin-packing pass that keeps every bin <= C_sub*128 slots.
#   2. DVE builds one-hot segment matrices S^T[e,d] = (rel[e]==d) from
#      host-provided relative-dest values via tensor_tensor(is_equal).
#   3. PE accumulates AGG^T = sum_j M_j^T @ S^T_j in PSUM per 128-dest
#      tile (bf16 matmuls, 1 cycle/row), then OUT^T = W^T @ AGG^T
#      + b (x) deg  (rank-1 bias matmul).
#   4. DVE scales by inv_deg along the dest axis; DMA writes OUT^T fp32.
# Host post-processing transposes and concatenates the per-core outputs.

import math

import numpy as np

P = 128
F = 128


class _Cfg:
    def __init__(self, n_nodes, n_cores, n_chunks, n_tiles, group_tiles=8):
        self.NN = n_nodes
        self.NCORES = n_cores
        self.NDEST = n_nodes // n_cores
        self.NT = n_tiles
        assert self.NT * P >= self.NDEST
        self.NCH = n_chunks
        self.CH = math.ceil(n_nodes / n_chunks)
        assert self.CH <= 32768
        self.G = group_tiles


CFG = _Cfg(100000, 8, 4, 100, group_tiles=8)

# Pads become idx=-1 (descriptor-trimmed) only when gather calls align
# with bins; exp.py disables this for large-call experiments.
PAD_TRIM = True

_BUILD_CACHE = {}


def _pack_tiles(cfg, cnt):
    """Assign NDEST dests to NT tiles (<=128 each) minimizing the max
    per-(tile, chunk) edge count.  cnt: [NDEST, NCH] int per-chunk degree."""
    NT, NCH = cfg.NT, cfg.NCH
    NDEST = cfg.NDEST
    order_d = np.argsort(-cnt.max(axis=1), kind="stable")
    sums = np.zeros((NT, NCH), np.int64)
    counts = np.zeros(NT, np.int64)
    pos = np.empty(NDEST, np.int64)
    big = np.int64(1) << 40
    for d in order_d:
        score = (sums + cnt[d]).max(axis=1)
        score[counts >= P] = big
        t = int(np.argmin(score))
        pos[d] = t * P + counts[t]
        counts[t] += 1
        sums[t] += cnt[d]
    return pos


def _host_prep(cfg, x, row, col, W, b):
    NN, NCORES, NDEST, NT, NCH, CH = (
        cfg.NN, cfg.NCORES, cfg.NDEST, cfg.NT, cfg.NCH, cfg.CH)
    NE = row.shape[0]
    row = np.asarray(row).astype(np.int64)
    col = np.asarray(col).astype(np.int64)
    x = np.ascontiguousarray(np.asarray(x, dtype=np.float32))
    W = np.ascontiguousarray(np.asarray(W, dtype=np.float32))
    b = np.asarray(b, dtype=np.float32)

    deg = np.bincount(row, minlength=NN).astype(np.float32)
    invdeg = np.where(deg > 0, 1.0 / np.maximum(deg, 1.0), 0.0).astype(np.float32)

    core = row // NDEST
    r_in_core = row % NDEST
    chunk = col // CH
    idx16 = (col % CH).astype(np.int16)

    # perm[core, d_local] = permuted position (tile*128 + slot).
    perm = np.zeros((NCORES, NDEST), np.int64)
    for c in range(NCORES):
        m = core == c
        key = r_in_core[m] * NCH + chunk[m]
        cnt = np.bincount(key, minlength=NDEST * NCH).reshape(NDEST, NCH)
        perm[c] = _pack_tiles(cfg, cnt)
    tilei = perm[core, r_in_core] // P
    rel = (perm[core, r_in_core] % P).astype(np.float32)

    bin_key = (core * NT + tilei) * NCH + chunk
    nbins = NCORES * NT * NCH
    counts = np.bincount(bin_key, minlength=nbins)
    C_sub = max(1, int(math.ceil(counts.max() / P)))
    SLOT = C_sub * P

    order = np.argsort(bin_key, kind="stable")
    sk = bin_key[order]
    starts = np.concatenate([[0], np.cumsum(counts)[:-1]])
    rank = np.arange(NE, dtype=np.int64) - starts[sk]
    pos = sk * SLOT + rank

    # pads get idx=-1 when bins align with the 1024-idx gather calls
    # (pads then trail within their call and the SWDGE ucode trims them,
    # generating no descriptors).  Mid-call negatives would fault, so use
    # idx=0 when calls cross bin boundaries.
    TOT = nbins * SLOT
    pad_idx = -1 if (PAD_TRIM and SLOT % 1024 == 0) else 0
    idx_pad = np.full(TOT, pad_idx, np.int16)
    rel_pad = np.full(TOT, -1.0, np.float32)
    idx_pad[pos] = idx16[order]
    rel_pad[pos] = rel[order]
    idx_pad = idx_pad.reshape(NCORES, NT, NCH, SLOT)
    rel_pad = rel_pad.reshape(NCORES, NT, NCH, C_sub, P)

    groups = [(t0, min(t0 + cfg.G, NT)) for t0 in range(0, NT, cfg.G)]

    import ml_dtypes
    bf16 = np.dtype(ml_dtypes.bfloat16)
    x_bf = np.ascontiguousarray(x.astype(bf16))
    W_bf = np.ascontiguousarray(W.astype(bf16))
    iota2 = np.tile(np.arange(P, dtype=np.float32)[None, :], (P, 1)).astype(bf16)
    brow = b[None, :].astype(bf16)

    in_maps = []
    for c in range(NCORES):
        # gather-call index stream: per (group, chunk), wrapped per
        # <=1024-idx call (SWDGE descriptor ring holds ~128 descs/queue;
        # a call generates num_idxs/16+1 per DMA engine), 16-wrapped and
        # replicated to 128 partitions.
        wrapped_parts = []
        for (t0, t1) in groups:
            for ch in range(NCH):
                seq = idx_pad[c, t0:t1, ch].reshape(-1)
                for k0 in range(0, len(seq), 1024):
                    seg = seq[k0:k0 + 1024]
                    wrapped_parts.append(np.tile(seg.reshape(-1, 16).T, (8, 1)))
        idx_t = np.concatenate(wrapped_parts, axis=1)

        # rel layout [P, (ch, t, j)] so a (group, chunk) slice is contiguous
        rel_t = np.ascontiguousarray(
            rel_pad[c].transpose(3, 1, 0, 2).reshape(P, NCH * NT * C_sub)
        ).astype(bf16)

        dsl = slice(c * NDEST, (c + 1) * NDEST)
        ivc = np.zeros(NT * P, np.float32)
        ivc[perm[c]] = invdeg[dsl]
        dgc = np.zeros(NT * P, np.float32)
        dgc[perm[c]] = deg[dsl]

        in_maps.append({
            "x": x_bf,
            "idxs": np.ascontiguousarray(idx_t),
            "rel": rel_t,
            "invdeg": np.ascontiguousarray(np.tile(ivc[None, :], (P, 1))),
            "degr": dgc[None, :].astype(bf16),
            "w": W_bf,
            "brow": brow,
            "iota2": iota2,
        })
    return C_sub, in_maps, perm


def _build(cfg, C_sub, repeat):
    import concourse.mybir as mybir
    import concourse.tile as tile
    from concourse import bacc

    f32 = mybir.dt.float32
    bf16 = mybir.dt.bfloat16
    i16 = mybir.dt.int16
    eq = mybir.AluOpType.is_equal
    mult = mybir.AluOpType.mult

    NT, NCH, CH, G = cfg.NT, cfg.NCH, cfg.CH, cfg.G
    IDXW = NT * NCH * C_sub * P // 16

    nc = bacc.Bacc("TRN2", debug=False, num_swdge_queues=4)
    x_d = nc.dram_tensor("x", [cfg.NN, F], bf16, kind="ExternalInput")
    idx_d = nc.dram_tensor("idxs", [P, IDXW], i16, kind="ExternalInput")
    rel_d = nc.dram_tensor("rel", [P, NCH * NT * C_sub], bf16, kind="ExternalInput")
    invdeg_d = nc.dram_tensor("invdeg", [P, NT * P], f32, kind="ExternalInput")
    deg_d = nc.dram_tensor("degr", [1, NT * P], bf16, kind="ExternalInput")
    w_d = nc.dram_tensor("w", [F, F], bf16, kind="ExternalInput")
    b_d = nc.dram_tensor("brow", [1, F], bf16, kind="ExternalInput")
    iota_d = nc.dram_tensor("iota2", [P, P], bf16, kind="ExternalInput")
    out_d = nc.dram_tensor("outT", [P, NT * P], f32, kind="ExternalOutput")

    groups = [(t0, min(t0 + G, NT)) for t0 in range(0, NT, G)]
    x_ap = x_d.ap()

    with tile.TileContext(nc) as tc:
        with (
            tc.tile_pool(name="const", bufs=1) as constp,
            tc.tile_pool(name="reg", bufs=2) as regionp,
            tc.tile_pool(name="st", bufs=2) as stp,
            tc.tile_pool(name="idx", bufs=2) as idxp,
            tc.tile_pool(name="small", bufs=8) as smallp,
            tc.tile_pool(name="grp", bufs=2) as grpp,
            # PSUM pools are bank-granular (8 banks of 2KB, one acc per
            # bank); with G=4 tiles/group, bufs=8 double-buffers two
            # groups of accumulators so group boundaries don't stall PE.
            tc.tile_pool(name="acc", bufs=8, space="PSUM") as accp,
        ):
            w_sb = constp.tile([F, F], bf16)
            nc.sync.dma_start(w_sb[:], w_d.ap())
            b_sb = constp.tile([1, F], bf16)
            nc.sync.dma_start(b_sb[:], b_d.ap())
            iota_sb = constp.tile([P, P], bf16)
            nc.sync.dma_start(iota_sb[:], iota_d.ap())
            rel_sb = constp.tile([P, NCH * NT * C_sub], bf16)
            nc.sync.dma_start(rel_sb[:], rel_d.ap())

            # Zero both reg pool buffers once: gather calls skip trailing
            # pad slots (idx=-1), leaving stale SBUF bytes there.  After
            # this, stale bytes are always finite x rows from a previous
            # batch (the one-hot zeroes their contribution, but NaN*0=NaN,
            # so the bytes must at least be finite).
            ncols0 = min(G, NT) * C_sub
            for _ in range(2):
                rz = regionp.tile([P, ncols0, P], bf16, tag="reg")
                nc.vector.memset(rz[:], 0.0)

            def body(_iv=None):
                idx_off = 0
                qn = 0
                for (t0, t1) in groups:
                    gt = t1 - t0
                    ncols = gt * C_sub
                    L = ncols * P
                    invdeg_g = grpp.tile([P, gt * P], f32, tag="invdeg")
                    nc.sync.dma_start(
                        invdeg_g[:], invdeg_d.ap()[:, t0 * P:t1 * P])
                    deg_g = grpp.tile([1, gt * P], bf16, tag="deg")
                    nc.sync.dma_start(deg_g[:], deg_d.ap()[:, t0 * P:t1 * P])
                    accs = [
                        accp.tile([P, P], f32, tag="acc",
                                  name=f"acc{t0}_{k}")[:]
                        for k in range(gt)
                    ]
                    for ch in range(NCH):
                        reg = regionp.tile([P, ncols, P], bf16, tag="reg")
                        idxt = idxp.tile([P, L // 16], i16, tag="idx")
                        nc.sync.dma_start(
                            idxt[:], idx_d.ap()[:, idx_off:idx_off + L // 16])
                        idx_off += L // 16
                        for k0 in range(0, ncols, 8):
                            kc = min(8, ncols - k0)
                            Lk = kc * P
                            nc.gpsimd.dma_gather(
                                out_ap=reg[:, k0:k0 + kc, :],
                                in_ap=x_ap[ch * CH:(ch + 1) * CH, :],
                                idxs_ap=idxt[:, k0 * 8:k0 * 8 + kc * 8],
                                num_idxs=Lk,
                                num_idxs_reg=Lk,
                                elem_size=F,
                                queue_num=qn % 4,
                            )
                            qn += 1
                        st = stp.tile([P, ncols, P], bf16, tag="st")
                        base = ch * NT * C_sub
                        rel_sl = rel_sb[:, base + t0 * C_sub:base + t1 * C_sub]
                        nc.vector.tensor_tensor(
                            out=st[:],
                            in0=iota_sb[:].unsqueeze(1).to_broadcast(
                                [P, ncols, P]),
                            in1=rel_sl.to_broadcast([P, ncols, P]),
                            op=eq,
                        )
                        for ti in range(gt):
                            accap = accs[ti]
                            for j in range(C_sub):
                                k = ti * C_sub + j
                                nc.tensor.matmul(
                                    out=accap,
                                    lhsT=reg[:, k, :],
                                    rhs=st[:, k, :],
                                    start=(ch == 0 and j == 0),
                                    stop=(ch == NCH - 1 and j == C_sub - 1),
                                )
                    for ti in range(gt):
                        t = t0 + ti
                        accap = accs[ti]
                        aggT = smallp.tile([P, P], bf16, tag="agg")
                        nc.scalar.copy(aggT[:], accap)
                        # reuse the same PSUM bank for the output matmul
                        nc.tensor.matmul(out=accap, lhsT=w_sb[:],
                                         rhs=aggT[:], start=True, stop=False)
                        nc.tensor.matmul(out=accap, lhsT=b_sb[:1, :],
                                         rhs=deg_g[:1, ti * P:(ti + 1) * P],
                                         start=False, stop=True)
                        osb = smallp.tile([P, P], f32, tag="osb")
                        nc.vector.tensor_tensor(
                            out=osb[:], in0=accap,
                            in1=invdeg_g[:, ti * P:(ti + 1) * P], op=mult)
                        nc.sync.dma_start(
                            out_d.ap()[:, t * P:(t + 1) * P], osb[:])

            if repeat == 1:
                body()
            else:
                with tc.For_i(0, repeat, 1) as iv:
                    body(iv)

    nc.compile()
    return nc


def _run(cfg, x, row, col, W, b, repeat=1, core_ids=None):
    from concourse import bass_utils

    C_sub, in_maps, perm = _host_prep(cfg, x, row, col, W, b)
    key = (cfg.NN, cfg.NCORES, C_sub, repeat)
    if key not in _BUILD_CACHE:
        _BUILD_CACHE[key] = _build(cfg, C_sub, repeat)
    nc = _BUILD_CACHE[key]
    if core_ids is None:
        core_ids = list(range(cfg.NCORES))
    res = bass_utils.run_bass_kernel_spmd(nc, in_maps, core_ids=core_ids)
    outs = []
    for c in range(len(core_ids)):
        outT = res.results[c]["outT"]
        outs.append(outT.T[perm[c]].astype(np.float32))
    return np.concatenate(outs, axis=0)


def kernel(x, row, col, W, b):
    return _run(CFG, x, row, col, W, b, repeat=1)


# revision 21
# speedup vs baseline: 1.0337x; 1.0147x over previous
# GNN mean-aggregation kernel for Trainium2 (8 NeuronCores, SPMD).
#
# Computes: out[i] = (1/deg_i) * sum_{(i,j) in E} (x[j] @ W + b)
# using the identity out = inv_deg * (A @ x) @ W + b*mask, so the dense
# linear layer runs once on the aggregated rows instead of per edge.
#
# Sharding: destination nodes (and their incoming edge rows -- `row` is
# sorted) are split contiguously across 8 cores; x and W are replicated,
# so no collectives are needed.
#
# Per-core pipeline (all payload data in bf16; PSUM accumulates fp32):
#   1. dma_gather (GPSIMD SWDGE) fetches x[col] rows (256B bf16 each)
#      from HBM, one call per (8-tile group, 25k-row source chunk), so
#      the ~1us fixed SWDGE cost amortizes over 8k indices.  int16
#      gather indices only span 32k rows, so x is addressed in 4 chunks
#      of 25k rows.  Edges are host-binned by (dest-tile, chunk) with a
#      vector bin-packing pass that keeps every bin <= C_sub*128 slots.
#   2. DVE builds one-hot segment matrices S^T[e,d] = (rel[e]==d) from
#      host-provided relative-dest values via tensor_tensor(is_equal).
#   3. PE accumulates AGG^T = sum_j M_j^T @ S^T_j in PSUM per 128-dest
#      tile (bf16 matmuls, 1 cycle/row), then OUT^T = W^T @ AGG^T
#      + b (x) deg  (rank-1 bias matmul).
#   4. DVE scales by inv_deg along the dest axis; DMA writes OUT^T fp32.
# Host post-processing transposes and concatenates the per-core outputs.

import math

import numpy as np

P = 128
F = 128


class _Cfg:
    def __init__(self, n_nodes, n_cores, n_chunks, n_tiles, group_tiles=8):
        self.NN = n_nodes
        self.NCORES = n_cores
        self.NDEST = n_nodes // n_cores
        self.NT = n_tiles
        assert self.NT * P >= self.NDEST
        self.NCH = n_chunks
        self.CH = math.ceil(n_nodes / n_chunks)
        assert self.CH <= 32768
        self.G = group_tiles


CFG = _Cfg(100000, 8, 4, 100, group_tiles=8)

# Pads become idx=-1 (descriptor-trimmed) only when gather calls align
# with bins; exp.py disables this for large-call experiments.
PAD_TRIM = True

_BUILD_CACHE = {}


def _pack_tiles(cfg, cnt):
    """Assign NDEST dests to NT tiles (<=128 each) minimizing the max
    per-(tile, chunk) edge count.  cnt: [NDEST, NCH] int per-chunk degree."""
    NT, NCH = cfg.NT, cfg.NCH
    NDEST = cfg.NDEST
    order_d = np.argsort(-cnt.max(axis=1), kind="stable")
    sums = np.zeros((NT, NCH), np.int64)
    counts = np.zeros(NT, np.int64)
    pos = np.empty(NDEST, np.int64)
    big = np.int64(1) << 40
    for d in order_d:
        score = (sums + cnt[d]).max(axis=1)
        score[counts >= P] = big
        t = int(np.argmin(score))
        pos[d] = t * P + counts[t]
        counts[t] += 1
        sums[t] += cnt[d]
    return pos


def _host_prep(cfg, x, row, col, W, b):
    NN, NCORES, NDEST, NT, NCH, CH = (
        cfg.NN, cfg.NCORES, cfg.NDEST, cfg.NT, cfg.NCH, cfg.CH)
    NE = row.shape[0]
    row = np.asarray(row).astype(np.int64)
    col = np.asarray(col).astype(np.int64)
    x = np.ascontiguousarray(np.asarray(x, dtype=np.float32))
    W = np.ascontiguousarray(np.asarray(W, dtype=np.float32))
    b = np.asarray(b, dtype=np.float32)

    deg = np.bincount(row, minlength=NN).astype(np.float32)
    invdeg = np.where(deg > 0, 1.0 / np.maximum(deg, 1.0), 0.0).astype(np.float32)

    core = row // NDEST
    r_in_core = row % NDEST
    chunk = col // CH
    idx16 = (col % CH).astype(np.int16)

    # perm[core, d_local] = permuted position (tile*128 + slot).
    perm = np.zeros((NCORES, NDEST), np.int64)
    for c in range(NCORES):
        m = core == c
        key = r_in_core[m] * NCH + chunk[m]
        cnt = np.bincount(key, minlength=NDEST * NCH).reshape(NDEST, NCH)
        perm[c] = _pack_tiles(cfg, cnt)
    tilei = perm[core, r_in_core] // P
    rel = (perm[core, r_in_core] % P).astype(np.float32)

    bin_key = (core * NT + tilei) * NCH + chunk
    nbins = NCORES * NT * NCH
    counts = np.bincount(bin_key, minlength=nbins)
    C_sub = max(1, int(math.ceil(counts.max() / P)))
    SLOT = C_sub * P

    order = np.argsort(bin_key, kind="stable")
    sk = bin_key[order]
    starts = np.concatenate([[0], np.cumsum(counts)[:-1]])
    rank = np.arange(NE, dtype=np.int64) - starts[sk]
    pos = sk * SLOT + rank

    # pads get idx=-1 when bins align with the 1024-idx gather calls
    # (pads then trail within their call and the SWDGE ucode trims them,
    # generating no descriptors).  Mid-call negatives would fault, so use
    # idx=0 when calls cross bin boundaries.
    TOT = nbins * SLOT
    pad_idx = -1 if (PAD_TRIM and SLOT % 1024 == 0) else 0
    idx_pad = np.full(TOT, pad_idx, np.int16)
    rel_pad = np.full(TOT, -1.0, np.float32)
    idx_pad[pos] = idx16[order]
    rel_pad[pos] = rel[order]
    idx_pad = idx_pad.reshape(NCORES, NT, NCH, SLOT)
    rel_pad = rel_pad.reshape(NCORES, NT, NCH, C_sub, P)

    groups = [(t0, min(t0 + cfg.G, NT)) for t0 in range(0, NT, cfg.G)]

    import ml_dtypes
    bf16 = np.dtype(ml_dtypes.bfloat16)
    x_bf = np.ascontiguousarray(x.astype(bf16))
    W_bf = np.ascontiguousarray(W.astype(bf16))
    iota2 = np.tile(np.arange(P, dtype=np.float32)[None, :], (P, 1)).astype(bf16)
    brow = b[None, :].astype(bf16)

    in_maps = []
    for c in range(NCORES):
        # gather-call index stream: per (group, chunk), wrapped per
        # <=1024-idx call (SWDGE descriptor ring holds ~128 descs/queue;
        # a call generates num_idxs/16+1 per DMA engine), 16-wrapped and
        # replicated to 128 partitions.
        wrapped_parts = []
        for (t0, t1) in groups:
            for ch in range(NCH):
                seq = idx_pad[c, t0:t1, ch].reshape(-1)
                for k0 in range(0, len(seq), 1024):
                    seg = seq[k0:k0 + 1024]
                    wrapped_parts.append(np.tile(seg.reshape(-1, 16).T, (8, 1)))
        idx_t = np.concatenate(wrapped_parts, axis=1)

        # rel layout [P, (ch, t, j)] so a (group, chunk) slice is contiguous
        rel_t = np.ascontiguousarray(
            rel_pad[c].transpose(3, 1, 0, 2).reshape(P, NCH * NT * C_sub)
        ).astype(bf16)

        dsl = slice(c * NDEST, (c + 1) * NDEST)
        ivc = np.zeros(NT * P, np.float32)
        ivc[perm[c]] = invdeg[dsl]
        dgc = np.zeros(NT * P, np.float32)
        dgc[perm[c]] = deg[dsl]

        in_maps.append({
            "x": x_bf,
            "idxs": np.ascontiguousarray(idx_t),
            "rel": rel_t,
            "invdeg": np.ascontiguousarray(np.tile(ivc[None, :], (P, 1))),
            "degr": dgc[None, :].astype(bf16),
            "w": W_bf,
            "brow": brow,
            "iota2": iota2,
        })
    return C_sub, in_maps, perm


def _build(cfg, C_sub, repeat):
    import concourse.mybir as mybir
    import concourse.tile as tile
    from concourse import bacc

    f32 = mybir.dt.float32
    bf16 = mybir.dt.bfloat16
    i16 = mybir.dt.int16
    eq = mybir.AluOpType.is_equal
    mult = mybir.AluOpType.mult

    NT, NCH, CH, G = cfg.NT, cfg.NCH, cfg.CH, cfg.G
    IDXW = NT * NCH * C_sub * P // 16

    nc = bacc.Bacc("TRN2", debug=False, num_swdge_queues=4)
    x_d = nc.dram_tensor("x", [cfg.NN, F], bf16, kind="ExternalInput")
    idx_d = nc.dram_tensor("idxs", [P, IDXW], i16, kind="ExternalInput")
    rel_d = nc.dram_tensor("rel", [P, NCH * NT * C_sub], bf16, kind="ExternalInput")
    invdeg_d = nc.dram_tensor("invdeg", [P, NT * P], f32, kind="ExternalInput")
    deg_d = nc.dram_tensor("degr", [1, NT * P], bf16, kind="ExternalInput")
    w_d = nc.dram_tensor("w", [F, F], bf16, kind="ExternalInput")
    b_d = nc.dram_tensor("brow", [1, F], bf16, kind="ExternalInput")
    iota_d = nc.dram_tensor("iota2", [P, P], bf16, kind="ExternalInput")
    out_d = nc.dram_tensor("outT", [P, NT * P], f32, kind="ExternalOutput")

    groups = [(t0, min(t0 + G, NT)) for t0 in range(0, NT, G)]
    x_ap = x_d.ap()

    with tile.TileContext(nc) as tc:
        with (
            tc.tile_pool(name="const", bufs=1) as constp,
            tc.tile_pool(name="reg", bufs=3) as regionp,
            tc.tile_pool(name="st", bufs=3) as stp,
            tc.tile_pool(name="idx", bufs=3) as idxp,
            tc.tile_pool(name="small", bufs=8) as smallp,
            tc.tile_pool(name="grp", bufs=2) as grpp,
            # PSUM pools are bank-granular (8 banks of 2KB, one acc per
            # bank); with G=4 tiles/group, bufs=8 double-buffers two
            # groups of accumulators so group boundaries don't stall PE.
            tc.tile_pool(name="acc", bufs=8, space="PSUM") as accp,
        ):
            w_sb = constp.tile([F, F], bf16)
            nc.sync.dma_start(w_sb[:], w_d.ap())
            b_sb = constp.tile([1, F], bf16)
            nc.sync.dma_start(b_sb[:], b_d.ap())
            iota_sb = constp.tile([P, P], bf16)
            nc.sync.dma_start(iota_sb[:], iota_d.ap())
            rel_sb = constp.tile([P, NCH * NT * C_sub], bf16)
            nc.sync.dma_start(rel_sb[:], rel_d.ap())

            # Zero both reg pool buffers once: gather calls skip trailing
            # pad slots (idx=-1), leaving stale SBUF bytes there.  After
            # this, stale bytes are always finite x rows from a previous
            # batch (the one-hot zeroes their contribution, but NaN*0=NaN,
            # so the bytes must at least be finite).
            ncols0 = min(G, NT) * C_sub
            for _ in range(3):
                rz = regionp.tile([P, ncols0, P], bf16, tag="reg")
                nc.vector.memset(rz[:], 0.0)

            def body(_iv=None):
                idx_off = 0
                qn = 0
                for (t0, t1) in groups:
                    gt = t1 - t0
                    ncols = gt * C_sub
                    L = ncols * P
                    invdeg_g = grpp.tile([P, gt * P], f32, tag="invdeg")
                    nc.sync.dma_start(
                        invdeg_g[:], invdeg_d.ap()[:, t0 * P:t1 * P])
                    deg_g = grpp.tile([1, gt * P], bf16, tag="deg")
                    nc.sync.dma_start(deg_g[:], deg_d.ap()[:, t0 * P:t1 * P])
                    accs = [
                        accp.tile([P, P], f32, tag="acc",
                                  name=f"acc{t0}_{k}")[:]
                        for k in range(gt)
                    ]
                    for ch in range(NCH):
                        reg = regionp.tile([P, ncols, P], bf16, tag="reg")
                        idxt = idxp.tile([P, L // 16], i16, tag="idx")
                        nc.sync.dma_start(
                            idxt[:], idx_d.ap()[:, idx_off:idx_off + L // 16])
                        idx_off += L // 16
                        for k0 in range(0, ncols, 8):
                            kc = min(8, ncols - k0)
                            Lk = kc * P
                            nc.gpsimd.dma_gather(
                                out_ap=reg[:, k0:k0 + kc, :],
                                in_ap=x_ap[ch * CH:(ch + 1) * CH, :],
                                idxs_ap=idxt[:, k0 * 8:k0 * 8 + kc * 8],
                                num_idxs=Lk,
                                num_idxs_reg=Lk,
                                elem_size=F,
                                queue_num=qn % 4,
                            )
                            qn += 1
                        st = stp.tile([P, ncols, P], bf16, tag="st")
                        base = ch * NT * C_sub
                        rel_sl = rel_sb[:, base + t0 * C_sub:base + t1 * C_sub]
                        nc.vector.tensor_tensor(
                            out=st[:],
                            in0=iota_sb[:].unsqueeze(1).to_broadcast(
                                [P, ncols, P]),
                            in1=rel_sl.to_broadcast([P, ncols, P]),
                            op=eq,
                        )
                        for ti in range(gt):
                            accap = accs[ti]
                            for j in range(C_sub):
                                k = ti * C_sub + j
                                nc.tensor.matmul(
                                    out=accap,
                                    lhsT=reg[:, k, :],
                                    rhs=st[:, k, :],
                                    start=(ch == 0 and j == 0),
                                    stop=(ch == NCH - 1 and j == C_sub - 1),
                                )
                    for ti in range(gt):
                        t = t0 + ti
                        accap = accs[ti]
                        aggT = smallp.tile([P, P], bf16, tag="agg")
                        nc.scalar.copy(aggT[:], accap)
                        # reuse the same PSUM bank for the output matmul
                        nc.tensor.matmul(out=accap, lhsT=w_sb[:],
                                         rhs=aggT[:], start=True, stop=False)
                        nc.tensor.matmul(out=accap, lhsT=b_sb[:1, :],
                                         rhs=deg_g[:1, ti * P:(ti + 1) * P],
                                         start=False, stop=True)
                        osb = smallp.tile([P, P], f32, tag="osb")
                        nc.vector.tensor_tensor(
                            out=osb[:], in0=accap,
                            in1=invdeg_g[:, ti * P:(ti + 1) * P], op=mult)
                        nc.sync.dma_start(
                            out_d.ap()[:, t * P:(t + 1) * P], osb[:])

            if repeat == 1:
                body()
            else:
                with tc.For_i(0, repeat, 1) as iv:
                    body(iv)

    nc.compile()
    return nc


def _run(cfg, x, row, col, W, b, repeat=1, core_ids=None):
    from concourse import bass_utils

    C_sub, in_maps, perm = _host_prep(cfg, x, row, col, W, b)
    key = (cfg.NN, cfg.NCORES, C_sub, repeat)
    if key not in _BUILD_CACHE:
        _BUILD_CACHE[key] = _build(cfg, C_sub, repeat)
    nc = _BUILD_CACHE[key]
    if core_ids is None:
        core_ids = list(range(cfg.NCORES))
    res = bass_utils.run_bass_kernel_spmd(nc, in_maps, core_ids=core_ids)
    outs = []
    for c in range(len(core_ids)):
        outT = res.results[c]["outT"]
        outs.append(outT.T[perm[c]].astype(np.float32))
    return np.concatenate(outs, axis=0)


def kernel(x, row, col, W, b):
    return _run(CFG, x, row, col, W, b, repeat=1)


# revision 22
# speedup vs baseline: 1.0514x; 1.0171x over previous
# GNN mean-aggregation kernel for Trainium2 (8 NeuronCores, SPMD).
#
# Computes: out[i] = (1/deg_i) * sum_{(i,j) in E} (x[j] @ W + b)
# using the identity out = inv_deg * (A @ x) @ W + b*mask, so the dense
# linear layer runs once on the aggregated rows instead of per edge.
#
# Sharding: destination nodes (and their incoming edge rows -- `row` is
# sorted) are split contiguously across 8 cores; x and W are replicated,
# so no collectives are needed.
#
# Per-core pipeline (all payload data in bf16; PSUM accumulates fp32):
#   1. dma_gather (GPSIMD SWDGE) fetches x[col] rows (256B bf16 each)
#      from HBM, one call per (8-tile group, 25k-row source chunk), so
#      the ~1us fixed SWDGE cost amortizes over 8k indices.  int16
#      gather indices only span 32k rows, so x is addressed in 4 chunks
#      of 25k rows.  Edges are host-binned by (dest-tile, chunk) with a
#      vector bin-packing pass that keeps every bin <= C_sub*128 slots.
#   2. DVE builds one-hot segment matrices S^T[e,d] = (rel[e]==d) from
#      host-provided relative-dest values via tensor_tensor(is_equal).
#   3. PE accumulates AGG^T = sum_j M_j^T @ S^T_j in PSUM per 128-dest
#      tile (bf16 matmuls, 1 cycle/row), then OUT^T = W^T @ AGG^T
#      + b (x) deg  (rank-1 bias matmul).
#   4. DVE scales by inv_deg along the dest axis; DMA writes OUT^T fp32.
# Host post-processing transposes and concatenates the per-core outputs.

import math

import numpy as np

P = 128
F = 128


class _Cfg:
    def __init__(self, n_nodes, n_cores, n_chunks, n_tiles, group_tiles=8):
        self.NN = n_nodes
        self.NCORES = n_cores
        self.NDEST = n_nodes // n_cores
        self.NT = n_tiles
        assert self.NT * P >= self.NDEST
        self.NCH = n_chunks
        self.CH = math.ceil(n_nodes / n_chunks)
        assert self.CH <= 32768
        self.G = group_tiles


CFG = _Cfg(100000, 8, 4, 100, group_tiles=8)

# Pads become idx=-1 (descriptor-trimmed) only when gather calls align
# with bins; exp.py disables this for large-call experiments.
PAD_TRIM = False

_BUILD_CACHE = {}


def _pack_tiles(cfg, cnt):
    """Assign NDEST dests to NT tiles (<=128 each) minimizing the max
    per-(tile, chunk) edge count.  cnt: [NDEST, NCH] int per-chunk degree."""
    NT, NCH = cfg.NT, cfg.NCH
    NDEST = cfg.NDEST
    order_d = np.argsort(-cnt.max(axis=1), kind="stable")
    sums = np.zeros((NT, NCH), np.int64)
    counts = np.zeros(NT, np.int64)
    pos = np.empty(NDEST, np.int64)
    big = np.int64(1) << 40
    for d in order_d:
        score = (sums + cnt[d]).max(axis=1)
        score[counts >= P] = big
        t = int(np.argmin(score))
        pos[d] = t * P + counts[t]
        counts[t] += 1
        sums[t] += cnt[d]
    return pos


def _host_prep(cfg, x, row, col, W, b):
    NN, NCORES, NDEST, NT, NCH, CH = (
        cfg.NN, cfg.NCORES, cfg.NDEST, cfg.NT, cfg.NCH, cfg.CH)
    NE = row.shape[0]
    row = np.asarray(row).astype(np.int64)
    col = np.asarray(col).astype(np.int64)
    x = np.ascontiguousarray(np.asarray(x, dtype=np.float32))
    W = np.ascontiguousarray(np.asarray(W, dtype=np.float32))
    b = np.asarray(b, dtype=np.float32)

    deg = np.bincount(row, minlength=NN).astype(np.float32)
    invdeg = np.where(deg > 0, 1.0 / np.maximum(deg, 1.0), 0.0).astype(np.float32)

    core = row // NDEST
    r_in_core = row % NDEST
    chunk = col // CH
    idx16 = (col % CH).astype(np.int16)

    # perm[core, d_local] = permuted position (tile*128 + slot).
    perm = np.zeros((NCORES, NDEST), np.int64)
    for c in range(NCORES):
        m = core == c
        key = r_in_core[m] * NCH + chunk[m]
        cnt = np.bincount(key, minlength=NDEST * NCH).reshape(NDEST, NCH)
        perm[c] = _pack_tiles(cfg, cnt)
    tilei = perm[core, r_in_core] // P
    rel = (perm[core, r_in_core] % P).astype(np.float32)

    bin_key = (core * NT + tilei) * NCH + chunk
    nbins = NCORES * NT * NCH
    counts = np.bincount(bin_key, minlength=nbins)
    C_sub = max(1, int(math.ceil(counts.max() / P)))
    SLOT = C_sub * P

    order = np.argsort(bin_key, kind="stable")
    sk = bin_key[order]
    starts = np.concatenate([[0], np.cumsum(counts)[:-1]])
    rank = np.arange(NE, dtype=np.int64) - starts[sk]
    pos = sk * SLOT + rank

    # pads get idx=-1 when bins align with the 1024-idx gather calls
    # (pads then trail within their call and the SWDGE ucode trims them,
    # generating no descriptors).  Mid-call negatives would fault, so use
    # idx=0 when calls cross bin boundaries.
    TOT = nbins * SLOT
    pad_idx = -1 if (PAD_TRIM and SLOT % 1024 == 0) else 0
    idx_pad = np.full(TOT, pad_idx, np.int16)
    rel_pad = np.full(TOT, -1.0, np.float32)
    idx_pad[pos] = idx16[order]
    rel_pad[pos] = rel[order]
    idx_pad = idx_pad.reshape(NCORES, NT, NCH, SLOT)
    rel_pad = rel_pad.reshape(NCORES, NT, NCH, C_sub, P)

    groups = [(t0, min(t0 + cfg.G, NT)) for t0 in range(0, NT, cfg.G)]

    import ml_dtypes
    bf16 = np.dtype(ml_dtypes.bfloat16)
    x_bf = np.ascontiguousarray(x.astype(bf16))
    W_bf = np.ascontiguousarray(W.astype(bf16))
    iota2 = np.tile(np.arange(P, dtype=np.float32)[None, :], (P, 1)).astype(bf16)
    brow = b[None, :].astype(bf16)

    in_maps = []
    for c in range(NCORES):
        # gather-call index stream: per (group, chunk), wrapped per
        # <=1024-idx call (SWDGE descriptor ring holds ~128 descs/queue;
        # a call generates num_idxs/16+1 per DMA engine), 16-wrapped and
        # replicated to 128 partitions.
        wrapped_parts = []
        for (t0, t1) in groups:
            for ch in range(NCH):
                seq = idx_pad[c, t0:t1, ch].reshape(-1)
                for k0 in range(0, len(seq), 1024):
                    seg = seq[k0:k0 + 1024]
                    wrapped_parts.append(np.tile(seg.reshape(-1, 16).T, (8, 1)))
        idx_t = np.concatenate(wrapped_parts, axis=1)

        # rel layout [P, (ch, t, j)] so a (group, chunk) slice is contiguous
        rel_t = np.ascontiguousarray(
            rel_pad[c].transpose(3, 1, 0, 2).reshape(P, NCH * NT * C_sub)
        ).astype(bf16)

        dsl = slice(c * NDEST, (c + 1) * NDEST)
        ivc = np.zeros(NT * P, np.float32)
        ivc[perm[c]] = invdeg[dsl]
        dgc = np.zeros(NT * P, np.float32)
        dgc[perm[c]] = deg[dsl]

        in_maps.append({
            "x": x_bf,
            "idxs": np.ascontiguousarray(idx_t),
            "rel": rel_t,
            "invdeg": np.ascontiguousarray(np.tile(ivc[None, :], (P, 1))),
            "degr": dgc[None, :].astype(bf16),
            "w": W_bf,
            "brow": brow,
            "iota2": iota2,
        })
    return C_sub, in_maps, perm


def _build(cfg, C_sub, repeat):
    import concourse.mybir as mybir
    import concourse.tile as tile
    from concourse import bacc

    f32 = mybir.dt.float32
    bf16 = mybir.dt.bfloat16
    i16 = mybir.dt.int16
    eq = mybir.AluOpType.is_equal
    mult = mybir.AluOpType.mult

    NT, NCH, CH, G = cfg.NT, cfg.NCH, cfg.CH, cfg.G
    IDXW = NT * NCH * C_sub * P // 16

    nc = bacc.Bacc("TRN2", debug=False, num_swdge_queues=4)
    x_d = nc.dram_tensor("x", [cfg.NN, F], bf16, kind="ExternalInput")
    idx_d = nc.dram_tensor("idxs", [P, IDXW], i16, kind="ExternalInput")
    rel_d = nc.dram_tensor("rel", [P, NCH * NT * C_sub], bf16, kind="ExternalInput")
    invdeg_d = nc.dram_tensor("invdeg", [P, NT * P], f32, kind="ExternalInput")
    deg_d = nc.dram_tensor("degr", [1, NT * P], bf16, kind="ExternalInput")
    w_d = nc.dram_tensor("w", [F, F], bf16, kind="ExternalInput")
    b_d = nc.dram_tensor("brow", [1, F], bf16, kind="ExternalInput")
    iota_d = nc.dram_tensor("iota2", [P, P], bf16, kind="ExternalInput")
    out_d = nc.dram_tensor("outT", [P, NT * P], f32, kind="ExternalOutput")

    groups = [(t0, min(t0 + G, NT)) for t0 in range(0, NT, G)]
    x_ap = x_d.ap()

    with tile.TileContext(nc) as tc:
        with (
            tc.tile_pool(name="const", bufs=1) as constp,
            tc.tile_pool(name="reg", bufs=3) as regionp,
            tc.tile_pool(name="st", bufs=3) as stp,
            tc.tile_pool(name="idx", bufs=3) as idxp,
            tc.tile_pool(name="small", bufs=8) as smallp,
            tc.tile_pool(name="grp", bufs=2) as grpp,
            # PSUM pools are bank-granular (8 banks of 2KB, one acc per
            # bank); with G=4 tiles/group, bufs=8 double-buffers two
            # groups of accumulators so group boundaries don't stall PE.
            tc.tile_pool(name="acc", bufs=8, space="PSUM") as accp,
        ):
            w_sb = constp.tile([F, F], bf16)
            nc.sync.dma_start(w_sb[:], w_d.ap())
            b_sb = constp.tile([1, F], bf16)
            nc.sync.dma_start(b_sb[:], b_d.ap())
            iota_sb = constp.tile([P, P], bf16)
            nc.sync.dma_start(iota_sb[:], iota_d.ap())
            rel_sb = constp.tile([P, NCH * NT * C_sub], bf16)
            nc.sync.dma_start(rel_sb[:], rel_d.ap())

            # Zero both reg pool buffers once: gather calls skip trailing
            # pad slots (idx=-1), leaving stale SBUF bytes there.  After
            # this, stale bytes are always finite x rows from a previous
            # batch (the one-hot zeroes their contribution, but NaN*0=NaN,
            # so the bytes must at least be finite).
            ncols0 = min(G, NT) * C_sub
            for _ in range(3):
                rz = regionp.tile([P, ncols0, P], bf16, tag="reg")
                nc.vector.memset(rz[:], 0.0)

            def body(_iv=None):
                idx_off = 0
                qn = 0
                for (t0, t1) in groups:
                    gt = t1 - t0
                    ncols = gt * C_sub
                    L = ncols * P
                    invdeg_g = grpp.tile([P, gt * P], f32, tag="invdeg")
                    nc.sync.dma_start(
                        invdeg_g[:], invdeg_d.ap()[:, t0 * P:t1 * P])
                    deg_g = grpp.tile([1, gt * P], bf16, tag="deg")
                    nc.sync.dma_start(deg_g[:], deg_d.ap()[:, t0 * P:t1 * P])
                    accs = [
                        accp.tile([P, P], f32, tag="acc",
                                  name=f"acc{t0}_{k}")[:]
                        for k in range(gt)
                    ]
                    for ch in range(NCH):
                        reg = regionp.tile([P, ncols, P], bf16, tag="reg")
                        idxt = idxp.tile([P, L // 16], i16, tag="idx")
                        nc.sync.dma_start(
                            idxt[:], idx_d.ap()[:, idx_off:idx_off + L // 16])
                        idx_off += L // 16
                        for k0 in range(0, ncols, 8):
                            kc = min(8, ncols - k0)
                            Lk = kc * P
                            nc.gpsimd.dma_gather(
                                out_ap=reg[:, k0:k0 + kc, :],
                                in_ap=x_ap[ch * CH:(ch + 1) * CH, :],
                                idxs_ap=idxt[:, k0 * 8:k0 * 8 + kc * 8],
                                num_idxs=Lk,
                                num_idxs_reg=Lk,
                                elem_size=F,
                                queue_num=qn % 4,
                            )
                            qn += 1
                        st = stp.tile([P, ncols, P], bf16, tag="st")
                        base = ch * NT * C_sub
                        rel_sl = rel_sb[:, base + t0 * C_sub:base + t1 * C_sub]
                        nc.vector.tensor_tensor(
                            out=st[:],
                            in0=iota_sb[:].unsqueeze(1).to_broadcast(
                                [P, ncols, P]),
                            in1=rel_sl.to_broadcast([P, ncols, P]),
                            op=eq,
                        )
                        for ti in range(gt):
                            accap = accs[ti]
                            for j in range(C_sub):
                                k = ti * C_sub + j
                                nc.tensor.matmul(
                                    out=accap,
                                    lhsT=reg[:, k, :],
                                    rhs=st[:, k, :],
                                    start=(ch == 0 and j == 0),
                                    stop=(ch == NCH - 1 and j == C_sub - 1),
                                )
                    for ti in range(gt):
                        t = t0 + ti
                        accap = accs[ti]
                        aggT = smallp.tile([P, P], bf16, tag="agg")
                        nc.scalar.copy(aggT[:], accap)
                        # reuse the same PSUM bank for the output matmul
                        nc.tensor.matmul(out=accap, lhsT=w_sb[:],
                                         rhs=aggT[:], start=True, stop=False)
                        nc.tensor.matmul(out=accap, lhsT=b_sb[:1, :],
                                         rhs=deg_g[:1, ti * P:(ti + 1) * P],
                                         start=False, stop=True)
                        osb = smallp.tile([P, P], f32, tag="osb")
                        nc.vector.tensor_tensor(
                            out=osb[:], in0=accap,
                            in1=invdeg_g[:, ti * P:(ti + 1) * P], op=mult)
                        nc.sync.dma_start(
                            out_d.ap()[:, t * P:(t + 1) * P], osb[:])

            if repeat == 1:
                body()
            else:
                with tc.For_i(0, repeat, 1) as iv:
                    body(iv)

    nc.compile()
    return nc


def _run(cfg, x, row, col, W, b, repeat=1, core_ids=None):
    from concourse import bass_utils

    C_sub, in_maps, perm = _host_prep(cfg, x, row, col, W, b)
    key = (cfg.NN, cfg.NCORES, C_sub, repeat)
    if key not in _BUILD_CACHE:
        _BUILD_CACHE[key] = _build(cfg, C_sub, repeat)
    nc = _BUILD_CACHE[key]
    if core_ids is None:
        core_ids = list(range(cfg.NCORES))
    res = bass_utils.run_bass_kernel_spmd(nc, in_maps, core_ids=core_ids)
    outs = []
    for c in range(len(core_ids)):
        outT = res.results[c]["outT"]
        outs.append(outT.T[perm[c]].astype(np.float32))
    return np.concatenate(outs, axis=0)


def kernel(x, row, col, W, b):
    return _run(CFG, x, row, col, W, b, repeat=1)


# revision 28
# speedup vs baseline: 1.2555x; 1.1941x over previous
# GNN mean-aggregation kernel for Trainium2 (8 NeuronCores, SPMD).
#
# Computes: out[i] = (1/deg_i) * sum_{(i,j) in E} (x[j] @ W + b)
# using the identity out = inv_deg * (A @ x) @ W + b*mask, so the dense
# linear layer runs once on the aggregated rows instead of per edge.
#
# Sharding: destination nodes (and their incoming edge rows -- `row` is
# sorted) are split contiguously across 8 cores; x and W are replicated,
# so no collectives are needed.
#
# Per-core pipeline (all payload data in bf16; PSUM accumulates fp32):
#   1. dma_gather (GPSIMD SWDGE) fetches x[col] rows (256B bf16 each)
#      from HBM, one call per (8-tile group, 25k-row source chunk), so
#      the ~1us fixed SWDGE cost amortizes over 8k indices.  int16
#      gather indices only span 32k rows, so x is addressed in 4 chunks
#      of 25k rows.  Edges are host-binned by (dest-tile, chunk) with a
#      vector bin-packing pass that keeps every bin <= C_sub*128 slots.
#   2. DVE builds one-hot segment matrices S^T[e,d] = (rel[e]==d) from
#      host-provided relative-dest values via tensor_tensor(is_equal).
#   3. PE accumulates AGG^T = sum_j M_j^T @ S^T_j in PSUM per 128-dest
#      tile (bf16 matmuls, 1 cycle/row), then OUT^T = W^T @ AGG^T
#      + b (x) deg  (rank-1 bias matmul).
#   4. DVE scales by inv_deg along the dest axis; DMA writes OUT^T fp32.
# Host post-processing transposes and concatenates the per-core outputs.

import math

import numpy as np

P = 128
F = 128


class _Cfg:
    def __init__(self, n_nodes, n_cores, n_chunks, n_tiles, group_tiles=8):
        self.NN = n_nodes
        self.NCORES = n_cores
        self.NDEST = n_nodes // n_cores
        self.NT = n_tiles
        assert self.NT * P >= self.NDEST
        self.NCH = n_chunks
        self.CH = math.ceil(n_nodes / n_chunks)
        assert self.CH <= 32768
        self.G = group_tiles


CFG = _Cfg(100000, 8, 4, 100, group_tiles=8)

# Pads become idx=-1 (descriptor-trimmed) only when gather calls align
# with bins; exp.py disables this for large-call experiments.
PAD_TRIM = False

# One-hot built as st2[e, d, k] (slab-last) so every tensor_tensor operand
# has a stride-1 last dim, enabling the DVE 2-byte 2x fast path; the
# matmul then reads rhs slices st2[:, :, k] with a strided free dim.
ONEHOT_DK = True

_BUILD_CACHE = {}


def _pack_tiles(cfg, cnt):
    """Assign NDEST dests to NT tiles (<=128 each) minimizing the max
    per-(tile, chunk) edge count.  cnt: [NDEST, NCH] int per-chunk degree."""
    NT, NCH = cfg.NT, cfg.NCH
    NDEST = cfg.NDEST
    order_d = np.argsort(-cnt.max(axis=1), kind="stable")
    sums = np.zeros((NT, NCH), np.int64)
    counts = np.zeros(NT, np.int64)
    pos = np.empty(NDEST, np.int64)
    big = np.int64(1) << 40
    for d in order_d:
        score = (sums + cnt[d]).max(axis=1)
        score[counts >= P] = big
        t = int(np.argmin(score))
        pos[d] = t * P + counts[t]
        counts[t] += 1
        sums[t] += cnt[d]
    return pos


def _host_prep(cfg, x, row, col, W, b):
    NN, NCORES, NDEST, NT, NCH, CH = (
        cfg.NN, cfg.NCORES, cfg.NDEST, cfg.NT, cfg.NCH, cfg.CH)
    NE = row.shape[0]
    row = np.asarray(row).astype(np.int64)
    col = np.asarray(col).astype(np.int64)
    x = np.ascontiguousarray(np.asarray(x, dtype=np.float32))
    W = np.ascontiguousarray(np.asarray(W, dtype=np.float32))
    b = np.asarray(b, dtype=np.float32)

    deg = np.bincount(row, minlength=NN).astype(np.float32)
    invdeg = np.where(deg > 0, 1.0 / np.maximum(deg, 1.0), 0.0).astype(np.float32)

    core = row // NDEST
    r_in_core = row % NDEST
    chunk = col // CH
    idx16 = (col % CH).astype(np.int16)

    # perm[core, d_local] = permuted position (tile*128 + slot).
    perm = np.zeros((NCORES, NDEST), np.int64)
    for c in range(NCORES):
        m = core == c
        key = r_in_core[m] * NCH + chunk[m]
        cnt = np.bincount(key, minlength=NDEST * NCH).reshape(NDEST, NCH)
        perm[c] = _pack_tiles(cfg, cnt)
    tilei = perm[core, r_in_core] // P
    rel = (perm[core, r_in_core] % P).astype(np.float32)

    bin_key = (core * NT + tilei) * NCH + chunk
    nbins = NCORES * NT * NCH
    counts = np.bincount(bin_key, minlength=nbins)
    C_sub = max(1, int(math.ceil(counts.max() / P)))
    SLOT = C_sub * P

    order = np.argsort(bin_key, kind="stable")
    sk = bin_key[order]
    starts = np.concatenate([[0], np.cumsum(counts)[:-1]])
    rank = np.arange(NE, dtype=np.int64) - starts[sk]
    pos = sk * SLOT + rank

    # pads get idx=-1 when bins align with the 1024-idx gather calls
    # (pads then trail within their call and the SWDGE ucode trims them,
    # generating no descriptors).  Mid-call negatives would fault, so use
    # idx=0 when calls cross bin boundaries.
    TOT = nbins * SLOT
    pad_idx = -1 if (PAD_TRIM and SLOT % 1024 == 0) else 0
    idx_pad = np.full(TOT, pad_idx, np.int16)
    rel_pad = np.full(TOT, -1.0, np.float32)
    idx_pad[pos] = idx16[order]
    rel_pad[pos] = rel[order]
    idx_pad = idx_pad.reshape(NCORES, NT, NCH, SLOT)
    rel_pad = rel_pad.reshape(NCORES, NT, NCH, C_sub, P)

    groups = [(t0, min(t0 + cfg.G, NT)) for t0 in range(0, NT, cfg.G)]

    import ml_dtypes
    bf16 = np.dtype(ml_dtypes.bfloat16)
    x_bf = np.ascontiguousarray(x.astype(bf16))
    W_bf = np.ascontiguousarray(W.astype(bf16))
    iota2 = np.tile(np.arange(P, dtype=np.float32)[None, :], (P, 1)).astype(bf16)
    ncols0 = min(cfg.G, NT) * C_sub
    iotaf = np.ascontiguousarray(np.broadcast_to(
        np.arange(P, dtype=np.float32)[None, :, None],
        (P, P, ncols0)).reshape(P, P * ncols0)).astype(bf16)
    brow = b[None, :].astype(bf16)

    in_maps = []
    for c in range(NCORES):
        # gather-call index stream: per (group, chunk), wrapped per
        # <=1024-idx call (SWDGE descriptor ring holds ~128 descs/queue;
        # a call generates num_idxs/16+1 per DMA engine), 16-wrapped and
        # replicated to 128 partitions.
        wrapped_parts = []
        for (t0, t1) in groups:
            for ch in range(NCH):
                seq = idx_pad[c, t0:t1, ch].reshape(-1)
                for k0 in range(0, len(seq), 1024):
                    seg = seq[k0:k0 + 1024]
                    wrapped_parts.append(np.tile(seg.reshape(-1, 16).T, (8, 1)))
        idx_t = np.concatenate(wrapped_parts, axis=1)

        # rel layout [P, (ch, t, j)] so a (group, chunk) slice is contiguous
        rel_t = np.ascontiguousarray(
            rel_pad[c].transpose(3, 1, 0, 2).reshape(P, NCH * NT * C_sub)
        ).astype(bf16)

        dsl = slice(c * NDEST, (c + 1) * NDEST)
        ivc = np.zeros(NT * P, np.float32)
        ivc[perm[c]] = invdeg[dsl]
        dgc = np.zeros(NT * P, np.float32)
        dgc[perm[c]] = deg[dsl]

        in_maps.append({
            "x": x_bf,
            "idxs": np.ascontiguousarray(idx_t),
            "rel": rel_t,
            "invdeg": np.ascontiguousarray(np.tile(ivc[None, :], (P, 1))),
            "degr": dgc[None, :].astype(bf16),
            "w": W_bf,
            "brow": brow,
            "iota2": iota2,
            "iotaf": iotaf,
        })
    return C_sub, in_maps, perm


def _build(cfg, C_sub, repeat):
    import concourse.mybir as mybir
    import concourse.tile as tile
    from concourse import bacc

    f32 = mybir.dt.float32
    bf16 = mybir.dt.bfloat16
    i16 = mybir.dt.int16
    eq = mybir.AluOpType.is_equal
    mult = mybir.AluOpType.mult

    NT, NCH, CH, G = cfg.NT, cfg.NCH, cfg.CH, cfg.G
    IDXW = NT * NCH * C_sub * P // 16

    nc = bacc.Bacc("TRN2", debug=False, num_swdge_queues=4)
    x_d = nc.dram_tensor("x", [cfg.NN, F], bf16, kind="ExternalInput")
    idx_d = nc.dram_tensor("idxs", [P, IDXW], i16, kind="ExternalInput")
    rel_d = nc.dram_tensor("rel", [P, NCH * NT * C_sub], bf16, kind="ExternalInput")
    invdeg_d = nc.dram_tensor("invdeg", [P, NT * P], f32, kind="ExternalInput")
    deg_d = nc.dram_tensor("degr", [1, NT * P], bf16, kind="ExternalInput")
    w_d = nc.dram_tensor("w", [F, F], bf16, kind="ExternalInput")
    b_d = nc.dram_tensor("brow", [1, F], bf16, kind="ExternalInput")
    iota_d = nc.dram_tensor("iota2", [P, P], bf16, kind="ExternalInput")
    ncols0 = min(G, NT) * C_sub
    iotaf_d = nc.dram_tensor("iotaf", [P, P * ncols0], bf16,
                             kind="ExternalInput")
    out_d = nc.dram_tensor("outT", [P, NT * P], f32, kind="ExternalOutput")

    groups = [(t0, min(t0 + G, NT)) for t0 in range(0, NT, G)]
    x_ap = x_d.ap()

    with tile.TileContext(nc) as tc:
        with (
            tc.tile_pool(name="const", bufs=1) as constp,
            tc.tile_pool(name="reg", bufs=3) as regionp,
            tc.tile_pool(name="st", bufs=3) as stp,
            tc.tile_pool(name="idx", bufs=3) as idxp,
            tc.tile_pool(name="small", bufs=8) as smallp,
            tc.tile_pool(name="grp", bufs=2) as grpp,
            # PSUM pools are bank-granular (8 banks of 2KB, one acc per
            # bank); with G=4 tiles/group, bufs=8 double-buffers two
            # groups of accumulators so group boundaries don't stall PE.
            tc.tile_pool(name="acc", bufs=8, space="PSUM") as accp,
        ):
            w_sb = constp.tile([F, F], bf16)
            nc.sync.dma_start(w_sb[:], w_d.ap())
            b_sb = constp.tile([1, F], bf16)
            nc.sync.dma_start(b_sb[:], b_d.ap())
            iota_sb = constp.tile([P, P], bf16)
            nc.sync.dma_start(iota_sb[:], iota_d.ap())
            iotaf_sb = constp.tile([P, P, ncols0], bf16)
            nc.sync.dma_start(iotaf_sb[:], iotaf_d.ap())
            rel_sb = constp.tile([P, NCH * NT * C_sub], bf16)
            nc.sync.dma_start(rel_sb[:], rel_d.ap())

            # Zero both reg pool buffers once: gather calls skip trailing
            # pad slots (idx=-1), leaving stale SBUF bytes there.  After
            # this, stale bytes are always finite x rows from a previous
            # batch (the one-hot zeroes their contribution, but NaN*0=NaN,
            # so the bytes must at least be finite).
            ncols0 = min(G, NT) * C_sub
            for _ in range(3):
                rz = regionp.tile([P, ncols0, P], bf16, tag="reg")
                nc.vector.memset(rz[:], 0.0)

            def body(_iv=None):
                idx_off = 0
                qn = 0
                for (t0, t1) in groups:
                    gt = t1 - t0
                    ncols = gt * C_sub
                    L = ncols * P
                    invdeg_g = grpp.tile([P, gt * P], f32, tag="invdeg")
                    nc.sync.dma_start(
                        invdeg_g[:], invdeg_d.ap()[:, t0 * P:t1 * P])
                    deg_g = grpp.tile([1, gt * P], bf16, tag="deg")
                    nc.sync.dma_start(deg_g[:], deg_d.ap()[:, t0 * P:t1 * P])
                    accs = [
                        accp.tile([P, P], f32, tag="acc",
                                  name=f"acc{t0}_{k}")[:]
                        for k in range(gt)
                    ]
                    for ch in range(NCH):
                        reg = regionp.tile([P, ncols, P], bf16, tag="reg")
                        idxt = idxp.tile([P, L // 16], i16, tag="idx")
                        nc.sync.dma_start(
                            idxt[:], idx_d.ap()[:, idx_off:idx_off + L // 16])
                        idx_off += L // 16
                        for k0 in range(0, ncols, 8):
                            kc = min(8, ncols - k0)
                            Lk = kc * P
                            nc.gpsimd.dma_gather(
                                out_ap=reg[:, k0:k0 + kc, :],
                                in_ap=x_ap[ch * CH:(ch + 1) * CH, :],
                                idxs_ap=idxt[:, k0 * 8:k0 * 8 + kc * 8],
                                num_idxs=Lk,
                                num_idxs_reg=Lk,
                                elem_size=F,
                                queue_num=qn % 4,
                            )
                            qn += 1
                        base = ch * NT * C_sub
                        rel_sl = rel_sb[:, base + t0 * C_sub:base + t1 * C_sub]
                        if ONEHOT_DK:
                            # slab-last layout: all operands keep a
                            # stride-1 last dim -> DVE 2x fast path
                            st = stp.tile([P, P, ncols], bf16, tag="st")
                            nc.vector.tensor_tensor(
                                out=st[:],
                                in0=iotaf_sb[:, :, 0:ncols],
                                in1=rel_sl.unsqueeze(1).to_broadcast(
                                    [P, P, ncols]),
                                op=eq,
                            )
                        else:
                            st = stp.tile([P, ncols, P], bf16, tag="st")
                            nc.vector.tensor_tensor(
                                out=st[:],
                                in0=iota_sb[:].unsqueeze(1).to_broadcast(
                                    [P, ncols, P]),
                                in1=rel_sl.to_broadcast([P, ncols, P]),
                                op=eq,
                            )
                        for ti in range(gt):
                            accap = accs[ti]
                            for j in range(C_sub):
                                k = ti * C_sub + j
                                rhs = (st[:, :, k] if ONEHOT_DK
                                       else st[:, k, :])
                                nc.tensor.matmul(
                                    out=accap,
                                    lhsT=reg[:, k, :],
                                    rhs=rhs,
                                    start=(ch == 0 and j == 0),
                                    stop=(ch == NCH - 1 and j == C_sub - 1),
                                )
                    for ti in range(gt):
                        t = t0 + ti
                        accap = accs[ti]
                        aggT = smallp.tile([P, P], bf16, tag="agg")
                        nc.scalar.copy(aggT[:], accap)
                        # reuse the same PSUM bank for the output matmul
                        nc.tensor.matmul(out=accap, lhsT=w_sb[:],
                                         rhs=aggT[:], start=True, stop=False)
                        nc.tensor.matmul(out=accap, lhsT=b_sb[:1, :],
                                         rhs=deg_g[:1, ti * P:(ti + 1) * P],
                                         start=False, stop=True)
                        osb = smallp.tile([P, P], f32, tag="osb")
                        nc.vector.tensor_tensor(
                            out=osb[:], in0=accap,
                            in1=invdeg_g[:, ti * P:(ti + 1) * P], op=mult)
                        nc.sync.dma_start(
                            out_d.ap()[:, t * P:(t + 1) * P], osb[:])

            if repeat == 1:
                body()
            else:
                with tc.For_i(0, repeat, 1) as iv:
                    body(iv)

    nc.compile()
    return nc


def _run(cfg, x, row, col, W, b, repeat=1, core_ids=None):
    from concourse import bass_utils

    C_sub, in_maps, perm = _host_prep(cfg, x, row, col, W, b)
    key = (cfg.NN, cfg.NCORES, C_sub, repeat)
    if key not in _BUILD_CACHE:
        _BUILD_CACHE[key] = _build(cfg, C_sub, repeat)
    nc = _BUILD_CACHE[key]
    if core_ids is None:
        core_ids = list(range(cfg.NCORES))
    res = bass_utils.run_bass_kernel_spmd(nc, in_maps, core_ids=core_ids)
    outs = []
    for c in range(len(core_ids)):
        outT = res.results[c]["outT"]
        outs.append(outT.T[perm[c]].astype(np.float32))
    return np.concatenate(outs, axis=0)


def kernel(x, row, col, W, b):
    return _run(CFG, x, row, col, W, b, repeat=1)
